# revision 4
# baseline (speedup 1.0000x reference)
"""Trainium2 Bass kernel for nn_Attention_52166672777561.

Strategy (8 NeuronCores, tensor-parallel over heads, 3 heads/core):
  - Host: concat txt|img sequence, transpose X, slice + even/odd-permute QKV
    weight columns per core, fold RMSNorm gains into RoPE coefficient planes,
    cast to bf16.
  - Device per core: QKV projection (lhsT = X^T chunks stationary, rhs = fused
    [Wq|Wk|Wv] slices), fused RMSNorm+RoPE on q/k, PE-transpose q/k to head
    layout [dh, S], SDPA per head in transposed form (scores^T tiles ->
    exp on ScalarE -> P^T bf16 -> PV via lhsT=V), Z row-sums via ones-matmul,
    normalization folded into PV-psum evacuation, out-projection partials
    (lhsT = O^T chunks, rhs = Wo/Wao row-slices).
  - Host: sum the 8 partial (2304,3072) outputs, add biases, split img/txt.
"""
import sys

import numpy as np

if "/opt/trn_rl_repo" not in sys.path:
    sys.path.insert(0, "/opt/trn_rl_repo")

import ml_dtypes  # noqa: E402

import concourse.bass as bass  # noqa: E402
import concourse.bacc as bacc  # noqa: E402
import concourse.mybir as mybir  # noqa: E402
import concourse.tile as tile  # noqa: E402
from concourse.bass_utils import run_bass_kernel_spmd  # noqa: E402
from concourse.masks import make_identity  # noqa: E402

BF16 = ml_dtypes.bfloat16
EPS = 1e-5
S_TXT, S_IMG, S = 256, 2048, 2304
D, H, DH = 3072, 24, 128
NCORES = 8
HPC = 3            # heads per core
DLOC = HPC * DH    # 384
MT = S // 128      # 18 m-tiles (seq)
KT = D // 128      # 24 k-tiles (model dim)
SOFTMAX_SCALE = 1.0 / float(np.sqrt(DH))
QBS = [512, 512, 512, 512, 256]   # ragged q blocks over S=2304
NB = 6             # 512-wide output column blocks (3072)

F32 = mybir.dt.float32
BF = mybir.dt.bfloat16


def _build_program():
    nc = bacc.Bacc(
        "TRN2",
        target_bir_lowering=False,
        debug=False,
        enable_asserts=False,
        num_devices=NCORES,
    )
    # DRAM I/O (per-core shapes; SPMD with different data per core)
    xt = nc.dram_tensor("xt", [D, S], BF, kind="ExternalInput").ap()
    w_img = nc.dram_tensor("w_img", [D, 3 * DLOC], BF, kind="ExternalInput").ap()
    w_txt = nc.dram_tensor("w_txt", [D, 3 * DLOC], BF, kind="ExternalInput").ap()
    cqa = nc.dram_tensor("cqa", [S, DH], BF, kind="ExternalInput").ap()
    cqb = nc.dram_tensor("cqb", [S, DH], BF, kind="ExternalInput").ap()
    cka = nc.dram_tensor("cka", [S, DH], BF, kind="ExternalInput").ap()
    ckb = nc.dram_tensor("ckb", [S, DH], BF, kind="ExternalInput").ap()
    wo_r = nc.dram_tensor("wo_r", [DLOC, D], BF, kind="ExternalInput").ap()
    wao_r = nc.dram_tensor("wao_r", [DLOC, D], BF, kind="ExternalInput").ap()
    out = nc.dram_tensor("out", [S, D], F32, kind="ExternalOutput").ap()

    with tile.TileContext(nc) as tc:
        with (
            tc.tile_pool(name="const", bufs=1) as const_pool,
            tc.tile_pool(name="wres", bufs=1) as wres_pool,
            tc.tile_pool(name="wstream", bufs=3) as wstream_pool,
            tc.tile_pool(name="xts", bufs=6) as xts_pool,
            tc.tile_pool(name="cpl", bufs=8) as cpl_pool,
            tc.tile_pool(name="qkv", bufs=1) as qkv_pool,
            tc.tile_pool(name="work", bufs=2) as work_pool,
            tc.tile_pool(name="pt", bufs=1) as pt_pool,
            tc.tile_pool(name="ot", bufs=2) as ot_pool,
            tc.tile_pool(name="oev", bufs=3) as oev_pool,
            tc.tile_pool(name="ps", bufs=8, space="PSUM") as ps_pool,
        ):
            ident = const_pool.tile([128, 128], BF)
            make_identity(nc, ident[:])
            ones = const_pool.tile([128, 1], BF)
            nc.vector.memset(ones[:], 1.0)

            # resident tensors
            wimg_sb = wres_pool.tile([128, KT, 3 * DLOC], BF)   # 55.3KB/part
            for kt in range(KT):
                nc.sync.dma_start(
                    wimg_sb[:, kt, :], w_img[kt * 128:(kt + 1) * 128, :]
                )
            wo_sb = wres_pool.tile([128, HPC, D], BF)           # 18.4KB/part
            for h in range(HPC):
                nc.sync.dma_start(wo_sb[:, h, :], wo_r[h * 128:(h + 1) * 128, :])
            wao_sb = wres_pool.tile([128, HPC, D], BF)
            for h in range(HPC):
                nc.sync.dma_start(wao_sb[:, h, :], wao_r[h * 128:(h + 1) * 128, :])

            qt_all = qkv_pool.tile([128, HPC, S], BF)   # q^T: [dh, (h, s)]
            kt_all = qkv_pool.tile([128, HPC, S], BF)
            v_all = qkv_pool.tile([128, MT, DLOC], BF)  # v: [s%128, (s//128, hd)]

            def norm_rope_transpose(mt, pq, pk, pv, ca_t, cb_t, cka_t, ckb_t):
                # v evacuation on ScalarE (idle during phase 1)
                nc.scalar.copy(v_all[:, mt, :], pv[:])
                for (px, caT, cbT, dst) in ((pq, ca_t, cb_t, qt_all),
                                            (pk, cka_t, ckb_t, kt_all)):
                    # sum of squares per head -> [128, 3]
                    # (ScalarE: walrus rejects DVE tensor_tensor reading the
                    # same PSUM bank twice)
                    sq = work_pool.tile([128, DLOC], F32, tag="sq")
                    nc.scalar.square(sq[:], px[:])
                    ss = work_pool.tile([128, HPC], F32, tag="ss")
                    nc.vector.reduce_sum(
                        ss[:], sq[:].rearrange("p (h d) -> p h d", h=HPC),
                        axis=mybir.AxisListType.X,
                    )
                    nc.vector.tensor_scalar(
                        ss[:], ss[:], 1.0 / DH, EPS,
                        op0=mybir.AluOpType.mult, op1=mybir.AluOpType.add,
                    )
                    nc.vector.reciprocal(ss[:], ss[:])
                    rs = work_pool.tile([128, HPC], F32, tag="rs")
                    nc.scalar.sqrt(rs[:], ss[:])

                    # rope: out[p,h,s,j] = CA[p,(s,j)]*x[p,h,0,j] + CB[p,(s,j)]*x[p,h,1,j]
                    x4 = px[:].rearrange("p (h s j) -> p h s j", h=HPC, s=2)
                    xe = x4[:, :, 0:1, :].to_broadcast((128, HPC, 2, 64))
                    xo = x4[:, :, 1:2, :].to_broadcast((128, HPC, 2, 64))
                    ca4 = caT[:].rearrange("p (s j) -> p s j", s=2).unsqueeze(1) \
                        .to_broadcast((128, HPC, 2, 64))
                    cb4 = cbT[:].rearrange("p (s j) -> p s j", s=2).unsqueeze(1) \
                        .to_broadcast((128, HPC, 2, 64))
                    t1 = work_pool.tile([128, HPC, 2, 64], F32, tag="t1")
                    t2 = work_pool.tile([128, HPC, 2, 64], F32, tag="t2")
                    nc.vector.tensor_tensor(t1[:], xe, ca4, mybir.AluOpType.mult)
                    nc.vector.tensor_tensor(t2[:], xo, cb4, mybir.AluOpType.mult)
                    nc.vector.tensor_add(t1[:], t1[:], t2[:])
                    xf = work_pool.tile([128, DLOC], BF, tag="xf")
                    nc.vector.tensor_tensor(
                        xf[:].rearrange("p (h d) -> p h d", h=HPC), t1[:].rearrange("p h s j -> p h (s j)"),
                        rs[:].unsqueeze(2).to_broadcast((128, HPC, DH)),
                        mybir.AluOpType.mult,
                    )
                    # PE transpose per head -> [dh, m] layout
                    for h in range(HPC):
                        tp = ps_pool.tile([128, 128], BF, tag="ps")
                        nc.tensor.transpose(
                            tp[:], xf[:, h * DH:(h + 1) * DH], ident[:]
                        )
                        nc.vector.tensor_copy(
                            dst[:, h, mt * 128:(mt + 1) * 128], tp[:]
                        )

            def load_cplanes(mt):
                tiles = []
                for src in (cqa, cqb, cka, ckb):
                    t = cpl_pool.tile([128, DH], BF, tag="cpl")
                    nc.sync.dma_start(t[:], src[mt * 128:(mt + 1) * 128, :])
                    tiles.append(t)
                return tiles

            # ---------------- Phase 1a: txt rows (m-tiles 0,1), streamed w_txt
            ptiles = [
                [ps_pool.tile([128, DLOC], F32, tag="ps", name=f"ptxt{m}_{i}")
                 for i in range(3)] for m in range(2)
            ]
            xtiles_txt = [[None] * KT for _ in range(2)]
            for kt in range(KT):
                wt = wstream_pool.tile([128, 3 * DLOC], BF, tag="wtxt")
                nc.sync.dma_start(wt[:], w_txt[kt * 128:(kt + 1) * 128, :])
                for m in range(2):
                    xc = xts_pool.tile([128, 128], BF, tag="xt")
                    nc.sync.dma_start(
                        xc[:], xt[kt * 128:(kt + 1) * 128, m * 128:(m + 1) * 128]
                    )
                    for i in range(3):
                        nc.tensor.matmul(
                            ptiles[m][i][:], xc[:], wt[:, i * DLOC:(i + 1) * DLOC],
                            start=(kt == 0), stop=(kt == KT - 1),
                        )
            for m in range(2):
                cps = load_cplanes(m)
                norm_rope_transpose(m, ptiles[m][0], ptiles[m][1], ptiles[m][2], *cps)

            # ---------------- Phase 1b: img rows (m-tiles 2..17), resident w_img
            for mt in range(2, MT):
                pq = ps_pool.tile([128, DLOC], F32, tag="ps", name=f"pq{mt}")
                pk = ps_pool.tile([128, DLOC], F32, tag="ps", name=f"pk{mt}")
                pv = ps_pool.tile([128, DLOC], F32, tag="ps", name=f"pv{mt}")
                for kt in range(KT):
                    xc = xts_pool.tile([128, 128], BF, tag="xt")
                    nc.sync.dma_start(
                        xc[:], xt[kt * 128:(kt + 1) * 128, mt * 128:(mt + 1) * 128]
                    )
                    for i, pdst in enumerate((pq, pk, pv)):
                        nc.tensor.matmul(
                            pdst[:], xc[:], wimg_sb[:, kt, i * DLOC:(i + 1) * DLOC],
                            start=(kt == 0), stop=(kt == KT - 1),
                        )
                cps = load_cplanes(mt)
                norm_rope_transpose(mt, pq, pk, pv, *cps)

            # ---------------- Phase 2: SDPA + out-projection, per q-block
            q_off = 0
            for qbi, qs in enumerate(QBS):
                for h in range(HPC):
                    ptile = pt_pool.tile([128, MT, qs], BF, tag="pt")
                    ppv = ps_pool.tile([128, qs], F32, tag="ps", name=f"ppv{qbi}_{h}")
                    pz = ps_pool.tile([1, qs], F32, tag="ps", name=f"pz{qbi}_{h}")
                    for kt in range(MT):
                        pst = ps_pool.tile([128, qs], F32, tag="ps",
                                           name=f"pst{qbi}_{h}_{kt}")
                        nc.tensor.matmul(
                            pst[:], kt_all[:, h, kt * 128:(kt + 1) * 128],
                            qt_all[:, h, q_off:q_off + qs],
                            start=True, stop=True,
                        )
                        nc.scalar.activation(
                            ptile[:, kt, :], pst[:],
                            mybir.ActivationFunctionType.Exp,
                            scale=SOFTMAX_SCALE,
                        )
                        nc.tensor.matmul(
                            ppv[:], v_all[:, kt, h * DH:(h + 1) * DH],
                            ptile[:, kt, :],
                            start=(kt == 0), stop=(kt == MT - 1),
                        )
                        nc.tensor.matmul(
                            pz[:], ones[:, 0:1], ptile[:, kt, :],
                            start=(kt == 0), stop=(kt == MT - 1),
                        )
                    z_sb = work_pool.tile([1, qs], F32, tag="zsb")
                    nc.vector.tensor_copy(z_sb[:], pz[:])
                    zb = work_pool.tile([128, qs], F32, tag="zb")
                    nc.gpsimd.partition_broadcast(zb[:], z_sb[:])
                    rz = work_pool.tile([128, qs], F32, tag="rz")
                    nc.vector.reciprocal(rz[:], zb[:])
                    otile = ot_pool.tile([128, HPC, 512], BF, tag="ot",
                                         name=f"ot{qbi}") if h == 0 else otile
                    nc.vector.tensor_tensor(
                        otile[:, h, :qs], ppv[:], rz[:], mybir.AluOpType.mult
                    )
                # out-projection for this q block
                for qc in range(qs // 128):
                    gc = q_off // 128 + qc
                    w_use = wao_sb if gc < 2 else wo_sb
                    for nb in range(NB):
                        po = ps_pool.tile([128, 512], F32, tag="ps",
                                          name=f"po{qbi}_{qc}_{nb}")
                        for h in range(HPC):
                            nc.tensor.matmul(
                                po[:], otile[:, h, qc * 128:(qc + 1) * 128],
                                w_use[:, h, nb * 512:(nb + 1) * 512],
                                start=(h == 0), stop=(h == HPC - 1),
                            )
                        osb = oev_pool.tile([128, 512], F32, tag="osb")
                        nc.vector.tensor_copy(osb[:], po[:])
                        nc.sync.dma_start(
                            out[gc * 128:(gc + 1) * 128, nb * 512:(nb + 1) * 512],
                            osb[:],
                        )
                q_off += qs

    nc.compile()
    return nc


_CACHED_NC = None


def _get_nc():
    global _CACHED_NC
    if _CACHED_NC is None:
        _CACHED_NC = _build_program()
    return _CACHED_NC


def _prep_in_maps(inp):
    X = np.concatenate(
        [np.asarray(inp["encoder_hidden_states"][0], np.float32),
         np.asarray(inp["hidden_states"][0], np.float32)], 0)
    Xt = np.ascontiguousarray(X.T).astype(BF16)
    f = np.asarray(inp["image_rotary_emb"], np.float32)
    A = np.ascontiguousarray(f[:, 0, :, :, 0].transpose(0, 2, 1).reshape(S, 128))
    Bp = np.ascontiguousarray(f[:, 0, :, :, 1].transpose(0, 2, 1).reshape(S, 128))
    perm = np.concatenate([np.arange(0, 128, 2), np.arange(1, 128, 2)])

    def gain_planes(g_img, g_txt):
        ca = np.empty((S, 128), np.float32)
        cb = np.empty((S, 128), np.float32)
        ca[:S_TXT] = np.tile(np.asarray(g_txt)[0::2], 2)[None]
        ca[S_TXT:] = np.tile(np.asarray(g_img)[0::2], 2)[None]
        cb[:S_TXT] = np.tile(np.asarray(g_txt)[1::2], 2)[None]
        cb[S_TXT:] = np.tile(np.asarray(g_img)[1::2], 2)[None]
        return (A * ca).astype(BF16), (Bp * cb).astype(BF16)

    cqa, cqb = gain_planes(inp["gq"], inp["gaq"])
    cka, ckb = gain_planes(inp["gk"], inp["gak"])

    Wq, Wk, Wv = (np.asarray(inp[k], np.float32) for k in ("Wq", "Wk", "Wv"))
    Waq, Wak, Wav = (np.asarray(inp[k], np.float32) for k in ("Waq", "Wak", "Wav"))
    Wo, Wao = np.asarray(inp["Wo"], np.float32), np.asarray(inp["Wao"], np.float32)

    in_maps = []
    for c in range(NCORES):
        sl = slice(DLOC * c, DLOC * (c + 1))

        def slice_perm(W):
            Wc = W[:, sl].reshape(D, HPC, 128)
            return Wc[:, :, perm].reshape(D, DLOC)

        w_img = np.concatenate(
            [slice_perm(Wq), slice_perm(Wk), Wv[:, sl]], 1).astype(BF16)
        w_txt = np.concatenate(
            [slice_perm(Waq), slice_perm(Wak), Wav[:, sl]], 1).astype(BF16)
        in_maps.append({
            "xt": Xt,
            "w_img": np.ascontiguousarray(w_img),
            "w_txt": np.ascontiguousarray(w_txt),
            "cqa": cqa, "cqb": cqb, "cka": cka, "ckb": ckb,
            "wo_r": np.ascontiguousarray(Wo[sl]).astype(BF16),
            "wao_r": np.ascontiguousarray(Wao[sl]).astype(BF16),
        })
    return in_maps


def kernel(trace=False, **inputs):
    nc = _get_nc()
    in_maps = _prep_in_maps(inputs)
    res = run_bass_kernel_spmd(
        nc, in_maps, core_ids=list(range(NCORES)), trace=trace
    )
    tot = np.zeros((S, D), np.float64)
    for c in range(NCORES):
        tot += res.results[c]["out"].astype(np.float64)
    img = (tot[S_TXT:] + np.asarray(inp_b := inputs["bo"], np.float64)[None]).astype(np.float32)
    txt = (tot[:S_TXT] + np.asarray(inputs["bao"], np.float64)[None]).astype(np.float32)
    if trace:
        kernel._last_exec_time_ns = res.exec_time_ns
        kernel._last_results = res
    return img[None], txt[None]


# revision 13
# speedup vs baseline: 1.2087x; 1.2087x over previous
"""Trainium2 Bass kernel for nn_Attention_52166672777561.

Strategy (8 NeuronCores, tensor-parallel over heads, 3 heads/core):
  - Host: concat txt|img sequence, transpose X, slice + even/odd-permute QKV
    weight columns per core, fold RMSNorm gains into RoPE coefficient planes,
    cast to bf16.
  - Device per core: QKV projection (lhsT = X^T chunks stationary, rhs = fused
    [Wq|Wk|Wv] slices), fused RMSNorm+RoPE on q/k, PE-transpose q/k to head
    layout [dh, S], SDPA per head in transposed form (scores^T tiles ->
    exp on ScalarE -> P^T bf16 -> PV via lhsT=V), Z row-sums via ones-matmul,
    normalization folded into PV-psum evacuation, out-projection partials
    (lhsT = O^T chunks, rhs = Wo/Wao row-slices).
  - Host: sum the 8 partial (2304,3072) outputs, add biases, split img/txt.
"""
import sys

import numpy as np

if "/opt/trn_rl_repo" not in sys.path:
    sys.path.insert(0, "/opt/trn_rl_repo")

import ml_dtypes  # noqa: E402

import concourse.bass as bass  # noqa: E402
import concourse.bacc as bacc  # noqa: E402
import concourse.mybir as mybir  # noqa: E402
import concourse.tile as tile  # noqa: E402
from concourse.bass_utils import run_bass_kernel_spmd  # noqa: E402
from concourse.masks import make_identity  # noqa: E402

BF16 = ml_dtypes.bfloat16
EPS = 1e-5
S_TXT, S_IMG, S = 256, 2048, 2304
D, H, DH = 3072, 24, 128
NCORES = 8
HPC = 3            # heads per core
DLOC = HPC * DH    # 384
MT = S // 128      # 18 m-tiles (seq)
KT = D // 128      # 24 k-tiles (model dim)
SOFTMAX_SCALE = 1.0 / float(np.sqrt(DH))
QBS = [512, 512, 512, 512, 256]   # ragged q blocks over S=2304
NB = 6             # 512-wide output column blocks (3072)

F32 = mybir.dt.float32
BF = mybir.dt.bfloat16


def _build_program():
    nc = bacc.Bacc(
        "TRN2",
        target_bir_lowering=False,
        debug=False,
        enable_asserts=False,
        num_devices=NCORES,
    )
    # DRAM I/O (per-core shapes; SPMD with different data per core)
    xt = nc.dram_tensor("xt", [D, S], BF, kind="ExternalInput").ap()
    w_img = nc.dram_tensor("w_img", [D, 3 * DLOC], BF, kind="ExternalInput").ap()
    w_txt = nc.dram_tensor("w_txt", [D, 3 * DLOC], BF, kind="ExternalInput").ap()
    cqa = nc.dram_tensor("cqa", [S, DH], BF, kind="ExternalInput").ap()
    cqb = nc.dram_tensor("cqb", [S, DH], BF, kind="ExternalInput").ap()
    cka = nc.dram_tensor("cka", [S, DH], BF, kind="ExternalInput").ap()
    ckb = nc.dram_tensor("ckb", [S, DH], BF, kind="ExternalInput").ap()
    wo_r = nc.dram_tensor("wo_r", [DLOC, D], BF, kind="ExternalInput").ap()
    wao_r = nc.dram_tensor("wao_r", [DLOC, D], BF, kind="ExternalInput").ap()
    out = nc.dram_tensor("out", [S, D], F32, kind="ExternalOutput").ap()

    with tile.TileContext(nc) as tc:
        with (
            tc.tile_pool(name="const", bufs=1) as const_pool,
            tc.tile_pool(name="wres", bufs=1) as wres_pool,
            tc.tile_pool(name="wstream", bufs=3) as wstream_pool,
            tc.tile_pool(name="xts", bufs=6) as xts_pool,
            tc.tile_pool(name="cpl", bufs=8) as cpl_pool,
            tc.tile_pool(name="qkv", bufs=1) as qkv_pool,
            tc.tile_pool(name="work", bufs=2) as work_pool,
            tc.tile_pool(name="pt", bufs=1) as pt_pool,
            tc.tile_pool(name="ot", bufs=2) as ot_pool,
            tc.tile_pool(name="oev", bufs=3) as oev_pool,
            tc.tile_pool(name="ps", bufs=6, space="PSUM") as ps_pool,
        ):
            ident = const_pool.tile([128, 128], BF)
            make_identity(nc, ident[:])
            ones = const_pool.tile([128, 1], BF)
            nc.vector.memset(ones[:], 1.0)

            # resident tensors
            wimg_sb = wres_pool.tile([128, KT, 3 * DLOC], BF)   # 55.3KB/part
            for kt in range(KT):
                nc.sync.dma_start(
                    wimg_sb[:, kt, :], w_img[kt * 128:(kt + 1) * 128, :]
                )
            wo_sb = wres_pool.tile([128, HPC, D], BF)           # 18.4KB/part
            for h in range(HPC):
                nc.sync.dma_start(wo_sb[:, h, :], wo_r[h * 128:(h + 1) * 128, :])
            wao_sb = wres_pool.tile([128, HPC, D], BF)
            for h in range(HPC):
                nc.sync.dma_start(wao_sb[:, h, :], wao_r[h * 128:(h + 1) * 128, :])

            qt_all = qkv_pool.tile([128, HPC, S], BF)   # q^T: [dh, (h, s)]
            kt_all = qkv_pool.tile([128, HPC, S], BF)
            v_all = qkv_pool.tile([128, MT, DLOC], BF)  # v: [s%128, (s//128, hd)]

            def norm_rope_transpose(mt, pq, pk, pv, ca_t, cb_t, cka_t, ckb_t):
                # v evacuation on ScalarE (idle during phase 1)
                nc.scalar.copy(v_all[:, mt, :], pv[:])
                # Evacuate q/k PSUM to SBUF immediately (ScalarE) so the PSUM
                # banks free fast and the next m-tile's matmuls never stall;
                # the DVE chain below then runs from SBUF at 2x fp32 rate.
                qsb = work_pool.tile([128, DLOC], F32, tag="qsb")
                ksb = work_pool.tile([128, DLOC], F32, tag="ksb")
                nc.scalar.copy(qsb[:], pq[:])
                nc.scalar.copy(ksb[:], pk[:])
                for (px, caT, cbT, dst) in ((qsb, ca_t, cb_t, qt_all),
                                            (ksb, cka_t, ckb_t, kt_all)):
                    # sum of squares per head -> [128, 3] (square on ScalarE to
                    # keep DVE free for the rope chain)
                    sq = work_pool.tile([128, DLOC], F32, tag="sq")
                    nc.scalar.square(sq[:], px[:])
                    ss = work_pool.tile([128, HPC], F32, tag="ss")
                    nc.vector.reduce_sum(
                        ss[:], sq[:].rearrange("p (h d) -> p h d", h=HPC),
                        axis=mybir.AxisListType.X,
                    )
                    nc.vector.tensor_scalar(
                        ss[:], ss[:], 1.0 / DH, EPS,
                        op0=mybir.AluOpType.mult, op1=mybir.AluOpType.add,
                    )
                    nc.vector.reciprocal(ss[:], ss[:])
                    rs = work_pool.tile([128, HPC], F32, tag="rs")
                    nc.scalar.sqrt(rs[:], ss[:])

                    # rope: out[p,h,s,j] = CA[p,(s,j)]*x[p,h,0,j] + CB[p,(s,j)]*x[p,h,1,j]
                    x4 = px[:].rearrange("p (h s j) -> p h s j", h=HPC, s=2)
                    xe = x4[:, :, 0:1, :].to_broadcast((128, HPC, 2, 64))
                    xo = x4[:, :, 1:2, :].to_broadcast((128, HPC, 2, 64))
                    ca4 = caT[:].rearrange("p (s j) -> p s j", s=2).unsqueeze(1) \
                        .to_broadcast((128, HPC, 2, 64))
                    cb4 = cbT[:].rearrange("p (s j) -> p s j", s=2).unsqueeze(1) \
                        .to_broadcast((128, HPC, 2, 64))
                    t1 = work_pool.tile([128, HPC, 2, 64], F32, tag="t1")
                    t2 = work_pool.tile([128, HPC, 2, 64], F32, tag="t2")
                    nc.vector.tensor_tensor(t1[:], xe, ca4, mybir.AluOpType.mult)
                    nc.vector.tensor_tensor(t2[:], xo, cb4, mybir.AluOpType.mult)
                    nc.vector.tensor_add(t1[:], t1[:], t2[:])
                    xf = work_pool.tile([128, DLOC], BF, tag="xf")
                    nc.vector.tensor_tensor(
                        xf[:].rearrange("p (h d) -> p h d", h=HPC), t1[:].rearrange("p h s j -> p h (s j)"),
                        rs[:].unsqueeze(2).to_broadcast((128, HPC, DH)),
                        mybir.AluOpType.mult,
                    )
                    # PE transpose per head -> [dh, m] layout (own psum tag so
                    # the stack allocator never blocks accumulator tiles)
                    for h in range(HPC):
                        tp = ps_pool.tile([128, 128], BF, tag="tr", bufs=2)
                        nc.tensor.transpose(
                            tp[:], xf[:, h * DH:(h + 1) * DH], ident[:]
                        )
                        nc.vector.tensor_copy(
                            dst[:, h, mt * 128:(mt + 1) * 128], tp[:]
                        )

            def load_cplanes(mt):
                tiles = []
                for src in (cqa, cqb, cka, ckb):
                    t = cpl_pool.tile([128, DH], BF, tag="cpl")
                    nc.sync.dma_start(t[:], src[mt * 128:(mt + 1) * 128, :])
                    tiles.append(t)
                return tiles

            # ---------------- Phase 1a: txt rows (m-tiles 0,1), streamed w_txt
            ptiles = [
                [ps_pool.tile([128, DLOC], F32, tag="ps", name=f"ptxt{m}_{i}")
                 for i in range(3)] for m in range(2)
            ]
            for kg in range(KT // 3):
                xcs = []
                for m in range(2):
                    xc = xts_pool.tile([128, 3, 128], BF, tag="xt")
                    nc.sync.dma_start(
                        xc[:],
                        xt[kg * 384:(kg + 1) * 384, m * 128:(m + 1) * 128]
                        .rearrange("(g p) m -> p g m", p=128),
                    )
                    xcs.append(xc)
                for g in range(3):
                    kt = kg * 3 + g
                    wt = wstream_pool.tile([128, 3 * DLOC], BF, tag="wtxt")
                    nc.sync.dma_start(wt[:], w_txt[kt * 128:(kt + 1) * 128, :])
                    for m in range(2):
                        for i in range(3):
                            nc.tensor.matmul(
                                ptiles[m][i][:], xcs[m][:, g, :],
                                wt[:, i * DLOC:(i + 1) * DLOC],
                                start=(kt == 0), stop=(kt == KT - 1),
                            )
            for m in range(2):
                cps = load_cplanes(m)
                norm_rope_transpose(m, ptiles[m][0], ptiles[m][1], ptiles[m][2], *cps)

            # ---------------- Phase 1b: img rows (m-tiles 2..17), resident w_img
            for mt in range(2, MT):
                pq = ps_pool.tile([128, DLOC], F32, tag="ps", name=f"pq{mt}")
                pk = ps_pool.tile([128, DLOC], F32, tag="ps", name=f"pk{mt}")
                pv = ps_pool.tile([128, DLOC], F32, tag="ps", name=f"pv{mt}")
                for kg in range(KT // 3):
                    xc = xts_pool.tile([128, 3, 128], BF, tag="xt")
                    nc.sync.dma_start(
                        xc[:],
                        xt[kg * 384:(kg + 1) * 384, mt * 128:(mt + 1) * 128]
                        .rearrange("(g p) m -> p g m", p=128),
                    )
                    for g in range(3):
                        kt = kg * 3 + g
                        for i, pdst in enumerate((pq, pk, pv)):
                            nc.tensor.matmul(
                                pdst[:], xc[:, g, :],
                                wimg_sb[:, kt, i * DLOC:(i + 1) * DLOC],
                                start=(kt == 0), stop=(kt == KT - 1),
                            )
                cps = load_cplanes(mt)
                norm_rope_transpose(mt, pq, pk, pv, *cps)

            # ---------------- Phase 2: SDPA + out-projection, per q-block.
            # Out-projection of block i is emitted after SDPA of block i+1 so
            # its (serial) z-normalization chain never stalls the PE.
            def outproj(qs, q_off, otile):
                for qc in range(qs // 128):
                    gc = q_off // 128 + qc
                    w_use = wao_sb if gc < 2 else wo_sb
                    for nb in range(NB):
                        po = ps_pool.tile([128, 512], F32, tag="ps",
                                          name=f"po{gc}_{nb}")
                        for h in range(HPC):
                            nc.tensor.matmul(
                                po[:], otile[:, h, qc * 128:(qc + 1) * 128],
                                w_use[:, h, nb * 512:(nb + 1) * 512],
                                start=(h == 0), stop=(h == HPC - 1),
                            )
                        osb = oev_pool.tile([128, 512], F32, tag="osb")
                        nc.vector.tensor_copy(osb[:], po[:])
                        nc.sync.dma_start(
                            out[gc * 128:(gc + 1) * 128, nb * 512:(nb + 1) * 512],
                            osb[:],
                        )

            pending = None  # (qs, q_off, otile) of the previous q-block
            q_off = 0
            for qbi, qs in enumerate(QBS):
                otile = ot_pool.tile([128, HPC, 512], BF, tag="ot", name=f"ot{qbi}")
                for h in range(HPC):
                    ptile = pt_pool.tile([128, MT, qs], BF, tag="pt")
                    ppv = ps_pool.tile([128, qs], F32, tag="ps", name=f"ppv{qbi}_{h}")
                    pz = ps_pool.tile([1, qs], F32, tag="ps", name=f"pz{qbi}_{h}")
                    for kt in range(MT):
                        pst = ps_pool.tile([128, qs], F32, tag="ps",
                                           name=f"pst{qbi}_{h}_{kt}")
                        nc.tensor.matmul(
                            pst[:], kt_all[:, h, kt * 128:(kt + 1) * 128],
                            qt_all[:, h, q_off:q_off + qs],
                            start=True, stop=True,
                        )
                        nc.scalar.activation(
                            ptile[:, kt, :], pst[:],
                            mybir.ActivationFunctionType.Exp,
                            scale=SOFTMAX_SCALE,
                        )
                        nc.tensor.matmul(
                            ppv[:], v_all[:, kt, h * DH:(h + 1) * DH],
                            ptile[:, kt, :],
                            start=(kt == 0), stop=(kt == MT - 1),
                        )
                        nc.tensor.matmul(
                            pz[:], ones[:, 0:1], ptile[:, kt, :],
                            start=(kt == 0), stop=(kt == MT - 1),
                        )
                    z_sb = work_pool.tile([1, qs], F32, tag="zsb")
                    nc.vector.reciprocal(z_sb[:], pz[:])
                    rz = work_pool.tile([128, qs], F32, tag="rz")
                    nc.gpsimd.partition_broadcast(rz[:], z_sb[:])
                    nc.vector.tensor_tensor(
                        otile[:, h, :qs], ppv[:], rz[:], mybir.AluOpType.mult
                    )
                if pending is not None:
                    outproj(*pending)
                pending = (qs, q_off, otile)
                q_off += qs
            outproj(*pending)

    nc.compile()
    return nc


_CACHED_NC = None


def _get_nc():
    global _CACHED_NC
    if _CACHED_NC is None:
        _CACHED_NC = _build_program()
    return _CACHED_NC


def _prep_in_maps(inp):
    X = np.concatenate(
        [np.asarray(inp["encoder_hidden_states"][0], np.float32),
         np.asarray(inp["hidden_states"][0], np.float32)], 0)
    Xt = np.ascontiguousarray(X.T).astype(BF16)
    f = np.asarray(inp["image_rotary_emb"], np.float32)
    A = np.ascontiguousarray(f[:, 0, :, :, 0].transpose(0, 2, 1).reshape(S, 128))
    Bp = np.ascontiguousarray(f[:, 0, :, :, 1].transpose(0, 2, 1).reshape(S, 128))
    perm = np.concatenate([np.arange(0, 128, 2), np.arange(1, 128, 2)])

    def gain_planes(g_img, g_txt):
        ca = np.empty((S, 128), np.float32)
        cb = np.empty((S, 128), np.float32)
        ca[:S_TXT] = np.tile(np.asarray(g_txt)[0::2], 2)[None]
        ca[S_TXT:] = np.tile(np.asarray(g_img)[0::2], 2)[None]
        cb[:S_TXT] = np.tile(np.asarray(g_txt)[1::2], 2)[None]
        cb[S_TXT:] = np.tile(np.asarray(g_img)[1::2], 2)[None]
        return (A * ca).astype(BF16), (Bp * cb).astype(BF16)

    cqa, cqb = gain_planes(inp["gq"], inp["gaq"])
    cka, ckb = gain_planes(inp["gk"], inp["gak"])

    Wq, Wk, Wv = (np.asarray(inp[k], np.float32) for k in ("Wq", "Wk", "Wv"))
    Waq, Wak, Wav = (np.asarray(inp[k], np.float32) for k in ("Waq", "Wak", "Wav"))
    Wo, Wao = np.asarray(inp["Wo"], np.float32), np.asarray(inp["Wao"], np.float32)

    in_maps = []
    for c in range(NCORES):
        sl = slice(DLOC * c, DLOC * (c + 1))

        def slice_perm(W):
            Wc = W[:, sl].reshape(D, HPC, 128)
            return Wc[:, :, perm].reshape(D, DLOC)

        w_img = np.concatenate(
            [slice_perm(Wq), slice_perm(Wk), Wv[:, sl]], 1).astype(BF16)
        w_txt = np.concatenate(
            [slice_perm(Waq), slice_perm(Wak), Wav[:, sl]], 1).astype(BF16)
        in_maps.append({
            "xt": Xt,
            "w_img": np.ascontiguousarray(w_img),
            "w_txt": np.ascontiguousarray(w_txt),
            "cqa": cqa, "cqb": cqb, "cka": cka, "ckb": ckb,
            "wo_r": np.ascontiguousarray(Wo[sl]).astype(BF16),
            "wao_r": np.ascontiguousarray(Wao[sl]).astype(BF16),
        })
    return in_maps


def kernel(trace=False, **inputs):
    nc = _get_nc()
    in_maps = _prep_in_maps(inputs)
    res = run_bass_kernel_spmd(
        nc, in_maps, core_ids=list(range(NCORES)), trace=trace
    )
    tot = np.zeros((S, D), np.float64)
    for c in range(NCORES):
        tot += res.results[c]["out"].astype(np.float64)
    img = (tot[S_TXT:] + np.asarray(inp_b := inputs["bo"], np.float64)[None]).astype(np.float32)
    txt = (tot[:S_TXT] + np.asarray(inputs["bao"], np.float64)[None]).astype(np.float32)
    if trace:
        kernel._last_exec_time_ns = res.exec_time_ns
        kernel._last_results = res
    return img[None], txt[None]


# revision 18
# speedup vs baseline: 1.4055x; 1.1628x over previous
"""Trainium2 Bass kernel for nn_Attention_52166672777561.

Strategy (8 NeuronCores, tensor-parallel over heads, 3 heads/core):
  - Host: concat txt|img sequence, transpose X, slice + even/odd-permute QKV
    weight columns per core, fold RMSNorm gains into RoPE coefficient planes,
    cast to bf16.
  - Device per core: QKV projection (lhsT = X^T chunks stationary, rhs = fused
    [Wq|Wk|Wv] slices), fused RMSNorm+RoPE on q/k, PE-transpose q/k to head
    layout [dh, S], SDPA per head in transposed form (scores^T tiles ->
    exp on ScalarE -> P^T bf16 -> PV via lhsT=V), Z row-sums via ones-matmul,
    normalization folded into PV-psum evacuation, out-projection partials
    (lhsT = O^T chunks, rhs = Wo/Wao row-slices).
  - Host: sum the 8 partial (2304,3072) outputs, add biases, split img/txt.
"""
import sys

import numpy as np

if "/opt/trn_rl_repo" not in sys.path:
    sys.path.insert(0, "/opt/trn_rl_repo")

import ml_dtypes  # noqa: E402

import concourse.bass as bass  # noqa: E402
import concourse.bacc as bacc  # noqa: E402
import concourse.mybir as mybir  # noqa: E402
import concourse.tile as tile  # noqa: E402
from concourse.bass_utils import run_bass_kernel_spmd  # noqa: E402
from concourse.masks import make_identity  # noqa: E402

BF16 = ml_dtypes.bfloat16
EPS = 1e-5
S_TXT, S_IMG, S = 256, 2048, 2304
D, H, DH = 3072, 24, 128
NCORES = 8
HPC = 3            # heads per core
DLOC = HPC * DH    # 384
MT = S // 128      # 18 m-tiles (seq)
KT = D // 128      # 24 k-tiles (model dim)
SOFTMAX_SCALE = 1.0 / float(np.sqrt(DH))
QBS = [512, 512, 512, 512, 256]   # ragged q blocks over S=2304
NB = 6             # 512-wide output column blocks (3072)

F32 = mybir.dt.float32
BF = mybir.dt.bfloat16


def _build_program():
    nc = bacc.Bacc(
        "TRN2",
        target_bir_lowering=False,
        debug=False,
        enable_asserts=False,
        num_devices=NCORES,
    )
    # DRAM I/O (per-core shapes; SPMD with different data per core)
    xt = nc.dram_tensor("xt", [D, S], BF, kind="ExternalInput").ap()
    w_img = nc.dram_tensor("w_img", [D, 3 * DLOC], BF, kind="ExternalInput").ap()
    w_txt = nc.dram_tensor("w_txt", [D, 3 * DLOC], BF, kind="ExternalInput").ap()
    cqa = nc.dram_tensor("cqa", [S, DH], BF, kind="ExternalInput").ap()
    cqb = nc.dram_tensor("cqb", [S, DH], BF, kind="ExternalInput").ap()
    cka = nc.dram_tensor("cka", [S, DH], BF, kind="ExternalInput").ap()
    ckb = nc.dram_tensor("ckb", [S, DH], BF, kind="ExternalInput").ap()
    wo_r = nc.dram_tensor("wo_r", [DLOC, D], BF, kind="ExternalInput").ap()
    wao_r = nc.dram_tensor("wao_r", [DLOC, D], BF, kind="ExternalInput").ap()
    out = nc.dram_tensor("out", [S, D], F32, kind="ExternalOutput").ap()

    with tile.TileContext(nc) as tc:
        with (
            tc.tile_pool(name="const", bufs=1) as const_pool,
            tc.tile_pool(name="wres", bufs=1) as wres_pool,
            tc.tile_pool(name="wstream", bufs=3) as wstream_pool,
            tc.tile_pool(name="xts", bufs=6) as xts_pool,
            tc.tile_pool(name="cpl", bufs=8) as cpl_pool,
            tc.tile_pool(name="qkv", bufs=1) as qkv_pool,
            tc.tile_pool(name="work", bufs=2) as work_pool,
            tc.tile_pool(name="pt", bufs=1) as pt_pool,
            tc.tile_pool(name="ot", bufs=2) as ot_pool,
            tc.tile_pool(name="oev", bufs=3) as oev_pool,
            tc.tile_pool(name="ps", bufs=6, space="PSUM") as ps_pool,
        ):
            ident = const_pool.tile([128, 128], BF)
            make_identity(nc, ident[:])
            ones = const_pool.tile([128, 1], BF)
            nc.vector.memset(ones[:], 1.0)

            qt_all = qkv_pool.tile([128, HPC, S], BF)   # q^T: [dh, (h, s)]
            kt_all = qkv_pool.tile([128, HPC, S], BF)
            v_all = qkv_pool.tile([128, MT, DLOC], BF)  # v: [s%128, (s//128, hd)]

            def norm_rope_transpose(mt, pq, pk, pv, ca_t, cb_t, cka_t, ckb_t):
                # v evacuation on ScalarE (idle during phase 1)
                nc.scalar.copy(v_all[:, mt, :], pv[:])
                # Evacuate q/k PSUM to SBUF immediately (ScalarE) so the PSUM
                # banks free fast and the next m-tile's matmuls never stall;
                # the DVE chain below then runs from SBUF at 2x fp32 rate.
                qsb = work_pool.tile([128, DLOC], F32, tag="qsb")
                ksb = work_pool.tile([128, DLOC], F32, tag="ksb")
                nc.scalar.copy(qsb[:], pq[:])
                nc.scalar.copy(ksb[:], pk[:])
                for (px, caT, cbT, dst) in ((qsb, ca_t, cb_t, qt_all),
                                            (ksb, cka_t, ckb_t, kt_all)):
                    # sum of squares per head -> [128, 3] (square on ScalarE to
                    # keep DVE free for the rope chain)
                    sq = work_pool.tile([128, DLOC], F32, tag="sq")
                    nc.scalar.square(sq[:], px[:])
                    ss = work_pool.tile([128, HPC], F32, tag="ss")
                    nc.vector.reduce_sum(
                        ss[:], sq[:].rearrange("p (h d) -> p h d", h=HPC),
                        axis=mybir.AxisListType.X,
                    )
                    nc.vector.tensor_scalar(
                        ss[:], ss[:], 1.0 / DH, EPS,
                        op0=mybir.AluOpType.mult, op1=mybir.AluOpType.add,
                    )
                    nc.vector.reciprocal(ss[:], ss[:])
                    rs = work_pool.tile([128, HPC], F32, tag="rs")
                    nc.scalar.sqrt(rs[:], ss[:])

                    # rope: out[p,h,s,j] = CA[p,(s,j)]*x[p,h,0,j] + CB[p,(s,j)]*x[p,h,1,j]
                    x4 = px[:].rearrange("p (h s j) -> p h s j", h=HPC, s=2)
                    xe = x4[:, :, 0:1, :].to_broadcast((128, HPC, 2, 64))
                    xo = x4[:, :, 1:2, :].to_broadcast((128, HPC, 2, 64))
                    ca4 = caT[:].rearrange("p (s j) -> p s j", s=2).unsqueeze(1) \
                        .to_broadcast((128, HPC, 2, 64))
                    cb4 = cbT[:].rearrange("p (s j) -> p s j", s=2).unsqueeze(1) \
                        .to_broadcast((128, HPC, 2, 64))
                    t1 = work_pool.tile([128, HPC, 2, 64], F32, tag="t1")
                    t2 = work_pool.tile([128, HPC, 2, 64], F32, tag="t2")
                    nc.vector.tensor_tensor(t1[:], xe, ca4, mybir.AluOpType.mult)
                    nc.vector.tensor_tensor(t2[:], xo, cb4, mybir.AluOpType.mult)
                    nc.vector.tensor_add(t1[:], t1[:], t2[:])
                    xf = work_pool.tile([128, DLOC], BF, tag="xf")
                    nc.vector.tensor_tensor(
                        xf[:].rearrange("p (h d) -> p h d", h=HPC), t1[:].rearrange("p h s j -> p h (s j)"),
                        rs[:].unsqueeze(2).to_broadcast((128, HPC, DH)),
                        mybir.AluOpType.mult,
                    )
                    # PE transpose per head -> [dh, m] layout (own psum tag so
                    # the stack allocator never blocks accumulator tiles)
                    for h in range(HPC):
                        tp = ps_pool.tile([128, 128], BF, tag="tr", bufs=2)
                        nc.tensor.transpose(
                            tp[:], xf[:, h * DH:(h + 1) * DH], ident[:]
                        )
                        nc.vector.tensor_copy(
                            dst[:, h, mt * 128:(mt + 1) * 128], tp[:]
                        )

            def load_cplanes(mt):
                tiles = []
                for src in (cqa, cqb, cka, ckb):
                    t = cpl_pool.tile([128, DH], BF, tag="cpl")
                    nc.sync.dma_start(t[:], src[mt * 128:(mt + 1) * 128, :])
                    tiles.append(t)
                return tiles

            # ---------------- Phase 1a: txt rows (m-tiles 0,1), streamed w_txt
            ptiles = [
                [ps_pool.tile([128, DLOC], F32, tag="ps", name=f"ptxt{m}_{i}")
                 for i in range(3)] for m in range(2)
            ]
            for kg in range(KT // 3):
                xcs = []
                for m in range(2):
                    xc = xts_pool.tile([128, 3, 128], BF, tag="xt")
                    nc.sync.dma_start(
                        xc[:],
                        xt[kg * 384:(kg + 1) * 384, m * 128:(m + 1) * 128]
                        .rearrange("(g p) m -> p g m", p=128),
                    )
                    xcs.append(xc)
                for g in range(3):
                    kt = kg * 3 + g
                    wt = wstream_pool.tile([128, 3 * DLOC], BF, tag="wtxt")
                    nc.sync.dma_start(wt[:], w_txt[kt * 128:(kt + 1) * 128, :])
                    for m in range(2):
                        for i in range(3):
                            nc.tensor.matmul(
                                ptiles[m][i][:], xcs[m][:, g, :],
                                wt[:, i * DLOC:(i + 1) * DLOC],
                                start=(kt == 0), stop=(kt == KT - 1),
                            )
            for m in range(2):
                cps = load_cplanes(m)
                norm_rope_transpose(m, ptiles[m][0], ptiles[m][1], ptiles[m][2], *cps)

            # resident w_img: emitted after phase 1a so the txt-phase stream
            # DMAs win the queue and the PE starts within a few us
            wimg_sb = wres_pool.tile([128, KT, 3 * DLOC], BF)   # 55.3KB/part
            for kt in range(KT):
                nc.sync.dma_start(
                    wimg_sb[:, kt, :], w_img[kt * 128:(kt + 1) * 128, :]
                )

            # ---------------- Phase 1b: img rows (m-tiles 2..17), resident w_img
            for mt in range(2, MT):
                pq = ps_pool.tile([128, DLOC], F32, tag="ps", name=f"pq{mt}")
                pk = ps_pool.tile([128, DLOC], F32, tag="ps", name=f"pk{mt}")
                pv = ps_pool.tile([128, DLOC], F32, tag="ps", name=f"pv{mt}")
                for kg in range(KT // 3):
                    xc = xts_pool.tile([128, 3, 128], BF, tag="xt")
                    nc.sync.dma_start(
                        xc[:],
                        xt[kg * 384:(kg + 1) * 384, mt * 128:(mt + 1) * 128]
                        .rearrange("(g p) m -> p g m", p=128),
                    )
                    for g in range(3):
                        kt = kg * 3 + g
                        for i, pdst in enumerate((pq, pk, pv)):
                            nc.tensor.matmul(
                                pdst[:], xc[:, g, :],
                                wimg_sb[:, kt, i * DLOC:(i + 1) * DLOC],
                                start=(kt == 0), stop=(kt == KT - 1),
                            )
                cps = load_cplanes(mt)
                norm_rope_transpose(mt, pq, pk, pv, *cps)

            # out-projection weights: needed only in phase 2
            wo_sb = wres_pool.tile([128, HPC, D], BF)           # 18.4KB/part
            for h in range(HPC):
                nc.sync.dma_start(wo_sb[:, h, :], wo_r[h * 128:(h + 1) * 128, :])
            wao_sb = wres_pool.tile([128, HPC, D], BF)
            for h in range(HPC):
                nc.sync.dma_start(wao_sb[:, h, :], wao_r[h * 128:(h + 1) * 128, :])

            # ---------------- Phase 2: SDPA + out-projection, per q-block.
            # Out-projection of block i is emitted after SDPA of block i+1 so
            # its (serial) z-normalization chain never stalls the PE.
            def outproj(qs, q_off, otile):
                for qc in range(qs // 128):
                    gc = q_off // 128 + qc
                    w_use = wao_sb if gc < 2 else wo_sb
                    for nb in range(NB):
                        po = ps_pool.tile([128, 512], F32, tag="ps",
                                          name=f"po{gc}_{nb}")
                        for h in range(HPC):
                            nc.tensor.matmul(
                                po[:], otile[:, h, qc * 128:(qc + 1) * 128],
                                w_use[:, h, nb * 512:(nb + 1) * 512],
                                start=(h == 0), stop=(h == HPC - 1),
                            )
                        osb = oev_pool.tile([128, 512], F32, tag="osb")
                        # alternate evac engine: share the load DVE/ACT
                        if nb % 2 == 0:
                            nc.vector.tensor_copy(osb[:], po[:])
                        else:
                            nc.scalar.copy(osb[:], po[:])
                        nc.sync.dma_start(
                            out[gc * 128:(gc + 1) * 128, nb * 512:(nb + 1) * 512],
                            osb[:],
                        )

            pending = None  # (qs, q_off, otile) of the previous q-block
            q_off = 0
            for qbi, qs in enumerate(QBS):
                otile = ot_pool.tile([128, HPC, 512], BF, tag="ot", name=f"ot{qbi}")
                for h in range(HPC):
                    ptile = pt_pool.tile([128, MT, qs], BF, tag="pt")
                    ppv = ps_pool.tile([128, qs], F32, tag="ps", name=f"ppv{qbi}_{h}")
                    pz = ps_pool.tile([1, qs], F32, tag="ps", name=f"pz{qbi}_{h}")
                    for kt in range(MT):
                        # scores psum on the "tr" tag: decouples the score-MM
                        # lookahead from accumulator/out-proj slot churn
                        pst = ps_pool.tile([128, qs], F32, tag="tr", bufs=2,
                                           name=f"pst{qbi}_{h}_{kt}")
                        nc.tensor.matmul(
                            pst[:], kt_all[:, h, kt * 128:(kt + 1) * 128],
                            qt_all[:, h, q_off:q_off + qs],
                            start=True, stop=True,
                        )
                        nc.scalar.activation(
                            ptile[:, kt, :], pst[:],
                            mybir.ActivationFunctionType.Exp,
                            scale=SOFTMAX_SCALE,
                        )
                        nc.tensor.matmul(
                            ppv[:], v_all[:, kt, h * DH:(h + 1) * DH],
                            ptile[:, kt, :],
                            start=(kt == 0), stop=(kt == MT - 1),
                        )
                        nc.tensor.matmul(
                            pz[:], ones[:, 0:1], ptile[:, kt, :],
                            start=(kt == 0), stop=(kt == MT - 1),
                        )
                    z_sb = work_pool.tile([1, qs], F32, tag="zsb")
                    nc.vector.reciprocal(z_sb[:], pz[:])
                    rz = work_pool.tile([128, qs], F32, tag="rz")
                    nc.gpsimd.partition_broadcast(rz[:], z_sb[:])
                    nc.vector.tensor_tensor(
                        otile[:, h, :qs], ppv[:], rz[:], mybir.AluOpType.mult
                    )
                if pending is not None:
                    outproj(*pending)
                pending = (qs, q_off, otile)
                q_off += qs
            outproj(*pending)

    nc.compile()
    return nc


_CACHED_NC = None


def _get_nc():
    global _CACHED_NC
    if _CACHED_NC is None:
        _CACHED_NC = _build_program()
    return _CACHED_NC


def _prep_in_maps(inp):
    X = np.concatenate(
        [np.asarray(inp["encoder_hidden_states"][0], np.float32),
         np.asarray(inp["hidden_states"][0], np.float32)], 0)
    Xt = np.ascontiguousarray(X.T).astype(BF16)
    f = np.asarray(inp["image_rotary_emb"], np.float32)
    A = np.ascontiguousarray(f[:, 0, :, :, 0].transpose(0, 2, 1).reshape(S, 128))
    Bp = np.ascontiguousarray(f[:, 0, :, :, 1].transpose(0, 2, 1).reshape(S, 128))
    perm = np.concatenate([np.arange(0, 128, 2), np.arange(1, 128, 2)])

    def gain_planes(g_img, g_txt):
        ca = np.empty((S, 128), np.float32)
        cb = np.empty((S, 128), np.float32)
        ca[:S_TXT] = np.tile(np.asarray(g_txt)[0::2], 2)[None]
        ca[S_TXT:] = np.tile(np.asarray(g_img)[0::2], 2)[None]
        cb[:S_TXT] = np.tile(np.asarray(g_txt)[1::2], 2)[None]
        cb[S_TXT:] = np.tile(np.asarray(g_img)[1::2], 2)[None]
        return (A * ca).astype(BF16), (Bp * cb).astype(BF16)

    cqa, cqb = gain_planes(inp["gq"], inp["gaq"])
    cka, ckb = gain_planes(inp["gk"], inp["gak"])

    Wq, Wk, Wv = (np.asarray(inp[k], np.float32) for k in ("Wq", "Wk", "Wv"))
    Waq, Wak, Wav = (np.asarray(inp[k], np.float32) for k in ("Waq", "Wak", "Wav"))
    Wo, Wao = np.asarray(inp["Wo"], np.float32), np.asarray(inp["Wao"], np.float32)

    in_maps = []
    for c in range(NCORES):
        sl = slice(DLOC * c, DLOC * (c + 1))

        def slice_perm(W):
            Wc = W[:, sl].reshape(D, HPC, 128)
            return Wc[:, :, perm].reshape(D, DLOC)

        w_img = np.concatenate(
            [slice_perm(Wq), slice_perm(Wk), Wv[:, sl]], 1).astype(BF16)
        w_txt = np.concatenate(
            [slice_perm(Waq), slice_perm(Wak), Wav[:, sl]], 1).astype(BF16)
        in_maps.append({
            "xt": Xt,
            "w_img": np.ascontiguousarray(w_img),
            "w_txt": np.ascontiguousarray(w_txt),
            "cqa": cqa, "cqb": cqb, "cka": cka, "ckb": ckb,
            "wo_r": np.ascontiguousarray(Wo[sl]).astype(BF16),
            "wao_r": np.ascontiguousarray(Wao[sl]).astype(BF16),
        })
    return in_maps


def kernel(trace=False, **inputs):
    nc = _get_nc()
    in_maps = _prep_in_maps(inputs)
    res = run_bass_kernel_spmd(
        nc, in_maps, core_ids=list(range(NCORES)), trace=trace
    )
    tot = np.zeros((S, D), np.float64)
    for c in range(NCORES):
        tot += res.results[c]["out"].astype(np.float64)
    img = (tot[S_TXT:] + np.asarray(inp_b := inputs["bo"], np.float64)[None]).astype(np.float32)
    txt = (tot[:S_TXT] + np.asarray(inputs["bao"], np.float64)[None]).astype(np.float32)
    if trace:
        kernel._last_exec_time_ns = res.exec_time_ns
        kernel._last_results = res
    return img[None], txt[None]
